# revision 40
# baseline (speedup 1.0000x reference)
"""Multi-head self-attention on 8 Trainium2 NeuronCores.

Problem: B=4, S=2048, D=1024, H=16 heads (dk=64), torch-Linear style
projections (y = x @ W.T + b), softmax attention, output projection.

Sharding: 8 cores = 4 batches x 2 head-groups (8 heads each).

Per-core schedule (designed around the scalar engine being the global
bottleneck: exp of 33.5M scores = ~290us at 128 lanes / 1.2 GHz):

  phase 1 (projections, tensor):
    QT = (Wq_g/sqrt(dk)) @ x_b.T + bq_g/sqrt(dk)  [512, S]  features on partitions
    KT = Wk_g @ x_b.T                             [512, S]  (bk cancels in softmax)
    V  = x_b @ Wv_g.T + bv_g                      [S, 512]  keys on partitions,
         spread into v2[kb] = per-pair [V_even | ones | ones | V_odd] blocks
  phase 2 (attention), per head-pair pr, query stripe qh (512 q), key block kb:
    - scores: TWO row-tiled matmuls (tile_position (0,0)/(64,0), K=64 each)
      that run CONCURRENTLY on the PE array, writing one [128, 1024] PSUM
      tile (head-even | head-odd).
    - ONE exp activation over the full [128, 1024] tile (F=1024 amortizes
      the ~350-cycle activation overhead; scalar engine does nothing else).
    - PV: two full-width matmuls accumulating [out_h | den_h] via the ones
      columns of v2 (denominators ride free in the unused M slack).
    - normalization is LAZY: pv PSUM banks are released by two DVE copies,
      then swap-den-halves (DMA), reciprocal and scale run off the critical
      path.  This keeps the exp stream back-to-back.
  phase 3: out_partial = Wo_g @ onorm, interleaved into phase-2 slack.

  All projection / phase-3 matmul groups are emitted inside phase-2 stripe
  loops ("hooks") so the Tile scheduler fills the tensor engine's idle time
  (phase 2 needs only ~660ns/kb of PE vs the 1147ns exp period).

PSUM budget (8 banks): scores 2x[128,1024] (4) + pv0/pv1 (2) + proj (2).

Device dtypes: bf16 matmul operands, f32 PSUM/exp-input/normalization.
"""

import math

import numpy as np
import ml_dtypes

import concourse.bass as bass
import concourse.bacc as bacc_mod
import concourse.mybir as mybir
import concourse.tile as tile
from concourse.bass_utils import run_bass_kernel_spmd

BF16 = mybir.dt.bfloat16
F32 = mybir.dt.float32
I32 = mybir.dt.int32
AF = mybir.ActivationFunctionType

# Schraudolph fast-exp constants (mean-zero variant): exp(x) ~=
# bitcast_f32(int32(x * 2^23/ln2 + (127*2^23 - 486411))), rms err 1.8%.
# Used on EXP_DVE_KBS of the 16 key blocks per stripe to offload the
# scalar engine; error contribution ~= 1.8% * sqrt(4/16) ~= 0.9% rms.
A_EXP = float(2**23 / math.log(2))
B_EXP = float(127 * 2**23 - 486411.0)
EXP_DVE_KBS = ()

B, S, D, H = 4, 2048, 1024, 16
DK = D // H  # 64
NCORES = 8
GROUPS = 2  # tensor-parallel head groups
DG = D // GROUPS  # 512 features per group
P = 128
FT = DG // P  # 4 feature tiles per group == head pairs
KB = S // P  # 16 key blocks
DKB = D // P  # 8 contraction blocks for projections
QH = 512  # query stripe
NQH = S // QH  # 4
DT = D // P  # 8 output feature tiles


def build_attention_nc(seq: int = S) -> bass.Bass:
    nc = bacc_mod.Bacc("TRN2", num_devices=NCORES)
    xt_d = nc.declare_dram_parameter("xt", [D, seq], BF16, isOutput=False)
    wqt_d = nc.declare_dram_parameter("wqt", [D, DG], BF16, isOutput=False)
    wkt_d = nc.declare_dram_parameter("wkt", [D, DG], BF16, isOutput=False)
    wvt_d = nc.declare_dram_parameter("wvt", [D, DG], BF16, isOutput=False)
    wot_d = nc.declare_dram_parameter("wot", [DG, D], BF16, isOutput=False)
    bq_d = nc.declare_dram_parameter("bqs", [P, FT], F32, isOutput=False)
    out_d = nc.declare_dram_parameter("out", [D, seq], F32, isOutput=True)

    with tile.TileContext(nc) as tc:
        with tc.tile_pool(name="persist", bufs=1) as persist:
            # ---- small persistent tiles ----
            bq_sb = persist.tile([P, FT], F32, name="bq_sb")
            nc.sync.dma_start(bq_sb, bq_d[:, :])

            # activation-table warmup: load the exp set before phase 2
            wsrc = persist.tile([1, 16], F32, name="wsrc")
            nc.vector.memset(wsrc, 0.0)
            wdst = persist.tile([1, 16], BF16, name="wdst")
            nc.scalar.activation(wdst, wsrc, AF.Exp)

            qt_sb = [persist.tile([P, seq], BF16, name=f"qt{i}") for i in range(FT)]
            kt_sb = [persist.tile([P, seq], BF16, name=f"kt{i}") for i in range(FT)]
            # v2[kb] per 256-col pair block: [V_even 64 | ones 128 | V_odd 64]
            v2_sb = [persist.tile([P, 2 * DG], BF16, name=f"v{i}") for i in range(KB)]
            wot_sb = [persist.tile([P, D], BF16, name=f"wot{i}") for i in range(FT)]
            onorm = [persist.tile([P, seq], BF16, name=f"onorm{i}") for i in range(FT)]

            # ---- input DMA (ordered so QT/KT(pr0,c0) can start earliest) ----
            with tc.tile_pool(name="xw", bufs=1) as xw_pool:
                xt_sb = [xw_pool.tile([P, seq], BF16, name=f"xts{i}") for i in range(DKB)]
                wqt_sb = [xw_pool.tile([P, DG], BF16, name=f"wqts{i}") for i in range(DKB)]
                wkt_sb = [xw_pool.tile([P, DG], BF16, name=f"wkts{i}") for i in range(DKB)]
                wvt_sb = [xw_pool.tile([P, DG], BF16, name=f"wvts{i}") for i in range(DKB)]
                csl0 = slice(0, QH)
                for k in range(DKB):
                    kslk = slice(k * P, (k + 1) * P)
                    nc.sync.dma_start(wqt_sb[k], wqt_d[kslk, :])
                    nc.sync.dma_start(wkt_sb[k], wkt_d[kslk, :])
                    nc.sync.dma_start(xt_sb[k][:, csl0], xt_d[kslk, csl0])
                    nc.sync.dma_start(wvt_sb[k], wvt_d[kslk, :])
                for c in range(1, NQH):
                    csl = slice(c * QH, (c + 1) * QH)
                    for k in range(DKB):
                        kslk = slice(k * P, (k + 1) * P)
                        nc.sync.dma_start(xt_sb[k][:, csl], xt_d[kslk, csl])
                for ft in range(FT):
                    nc.sync.dma_start(wot_sb[ft], wot_d[ft * P : (ft + 1) * P, :])

                with (
                    tc.tile_pool(name="pps", bufs=2, space="PSUM") as proj_ps,
                    tc.tile_pool(name="seps", bufs=2, space="PSUM") as se_ps,
                    tc.tile_pool(name="pvps", bufs=1, space="PSUM") as pv_ps,
                    tc.tile_pool(name="epool", bufs=8) as e_pool,
                    tc.tile_pool(name="ipool", bufs=2) as i32_pool,
                    tc.tile_pool(name="tpool", bufs=2) as tmp_pool,
                    tc.tile_pool(name="npool", bufs=2) as norm_pool,
                    tc.tile_pool(name="opool", bufs=4) as o_pool,
                    tc.tile_pool(name="p3pool", bufs=1) as p3_pool,
                ):
                    # ---------- emission helpers ----------
                    def proj_qk(pr, c, kind):
                        fsl = slice(pr * P, (pr + 1) * P)
                        csl = slice(c * QH, (c + 1) * QH)
                        ps = proj_ps.tile([P, QH], F32, name="psp", tag="proj")
                        w_sb = wqt_sb if kind == "q" else wkt_sb
                        for k in range(DKB):
                            nc.tensor.matmul(
                                ps,
                                lhsT=w_sb[k][:, fsl],
                                rhs=xt_sb[k][:, csl],
                                start=k == 0,
                                stop=k == DKB - 1,
                            )
                        if kind == "q":
                            # bias add folded into the PSUM->SBUF move
                            nc.vector.tensor_scalar(
                                qt_sb[pr][:, csl], ps, bq_sb[:, pr : pr + 1], None,
                                mybir.AluOpType.add,
                            )
                        else:
                            nc.vector.tensor_copy(kt_sb[pr][:, csl], ps)

                    def proj_v(kb):
                        # bv is folded into bo on the host (sum(p)==1), so V
                        # needs no bias here.
                        ksl = slice(kb * P, (kb + 1) * P)
                        ps = proj_ps.tile([P, DG], F32, name="psp", tag="proj")
                        for k in range(DKB):
                            nc.tensor.matmul(
                                ps,
                                lhsT=xt_sb[k][:, ksl],
                                rhs=wvt_sb[k],
                                start=k == 0,
                                stop=k == DKB - 1,
                            )
                        nc.vector.memset(v2_sb[kb], 1.0)
                        # even heads -> cols [256q, 256q+64); odd -> [256q+192, 256q+256)
                        nc.vector.tensor_copy(
                            v2_sb[kb].rearrange("p (q c) -> p q c", c=256)[:, :, 0:64],
                            ps.rearrange("p (q c) -> p q c", c=128)[:, :, 0:64],
                        )
                        nc.vector.tensor_copy(
                            v2_sb[kb].rearrange("p (q c) -> p q c", c=256)[:, :, 192:256],
                            ps.rearrange("p (q c) -> p q c", c=128)[:, :, 64:128],
                        )

                    def phase3_chunk(dt, c):
                        dsl = slice(dt * P, (dt + 1) * P)
                        csl = slice(c * QH, (c + 1) * QH)
                        ps = proj_ps.tile([P, QH], F32, name="psp", tag="proj")
                        for ft in range(FT):
                            nc.tensor.matmul(
                                ps,
                                lhsT=wot_sb[ft][:, dsl],
                                rhs=onorm[ft][:, csl],
                                start=ft == 0,
                                stop=ft == FT - 1,
                            )
                        o_sb = o_pool.tile([P, QH], F32, name="osb", tag="osb")
                        nc.vector.tensor_copy(o_sb, ps)
                        half = QH // 2
                        for h in range(2):
                            nc.sync.dma_start(
                                out_d[dsl, slice(csl.start + h * half, csl.start + (h + 1) * half)],
                                o_sb[:, h * half : (h + 1) * half],
                            )

                    # Last-stripe phase3 split: ft 0..2 pre-accumulate during
                    # stripe(3,3); only the ft=3 matmul + add follow the final
                    # normalize, shortening the tail.
                    p3_partial = {}

                    def phase3_pre(dt):
                        dsl = slice(dt * P, (dt + 1) * P)
                        csl = slice((NQH - 1) * QH, NQH * QH)
                        ps = proj_ps.tile([P, QH], F32, name="psp", tag="proj")
                        for ft in range(FT - 1):
                            nc.tensor.matmul(
                                ps,
                                lhsT=wot_sb[ft][:, dsl],
                                rhs=onorm[ft][:, csl],
                                start=ft == 0,
                                stop=ft == FT - 2,
                            )
                        part = p3_pool.tile([P, QH], BF16, name=f"p3p{dt}", tag=f"p3p{dt}")
                        nc.vector.tensor_copy(part, ps)
                        p3_partial[dt] = part

                    def phase3_post(dt):
                        dsl = slice(dt * P, (dt + 1) * P)
                        csl = slice((NQH - 1) * QH, NQH * QH)
                        ps = proj_ps.tile([P, QH], F32, name="psp", tag="proj")
                        nc.tensor.matmul(
                            ps,
                            lhsT=wot_sb[FT - 1][:, dsl],
                            rhs=onorm[FT - 1][:, csl],
                            start=True, stop=True,
                        )
                        o_sb = o_pool.tile([P, QH], F32, name="osb", tag="osb")
                        nc.vector.tensor_tensor(
                            o_sb, ps, p3_partial[dt], mybir.AluOpType.add
                        )
                        half = QH // 2
                        for h in range(2):
                            nc.sync.dma_start(
                                out_d[dsl, slice(csl.start + h * half, csl.start + (h + 1) * half)],
                                o_sb[:, h * half : (h + 1) * half],
                            )

                    def stripe(pr, qh, hooks):
                        """16-kb attention stripe; hooks: {kb: [thunk,...]}.

                        Emission order is scores(kb+1) BEFORE pv(kb): by the
                        time pv(kb) reaches the tensor queue head its exp(kb)
                        wait is already satisfied, so the fused LDWEIGHTS can
                        pipeline behind the previous matmul's stream instead
                        of serializing after the semaphore wait.
                        """
                        qsl = slice(qh * QH, (qh + 1) * QH)
                        h0c = slice(2 * pr * P, (2 * pr + 1) * P)
                        h1c = slice((2 * pr + 1) * P, (2 * pr + 2) * P)
                        pv0 = pv_ps.tile([P, QH], F32, name="pv0", tag="pv0")
                        pv1 = pv_ps.tile([P, QH], F32, name="pv1", tag="pv1")

                        def scores(kb):
                            ksl = slice(kb * P, (kb + 1) * P)
                            se = se_ps.tile([P, 2 * QH], F32, name="se", tag="se")
                            nc.tensor.matmul(
                                se[:, 0:QH],
                                lhsT=kt_sb[pr][0:64, ksl],
                                rhs=qt_sb[pr][0:64, qsl],
                                start=True, stop=True,
                            )
                            nc.tensor.matmul(
                                se[:, QH : 2 * QH],
                                lhsT=kt_sb[pr][64:128, ksl],
                                rhs=qt_sb[pr][64:128, qsl],
                                start=True, stop=True,
                            )
                            return se

                        se = scores(0)
                        for kb in range(KB):
                            e = e_pool.tile([P, 2 * QH], BF16, name="e", tag="e")
                            if kb in EXP_DVE_KBS:
                                # fast-exp on the vector engine (2 passes)
                                i32 = i32_pool.tile(
                                    [P, 2 * QH], I32, name="ei", tag="ei"
                                )
                                nc.vector.tensor_scalar(
                                    i32, se, A_EXP, B_EXP,
                                    mybir.AluOpType.mult, mybir.AluOpType.add,
                                )
                                nc.vector.tensor_copy(e, i32.bitcast(F32))
                            else:
                                nc.scalar.activation(e, se, AF.Exp)
                            if kb + 1 < KB:
                                se = scores(kb + 1)
                            first, last = kb == 0, kb == KB - 1
                            nc.tensor.matmul(
                                pv0, lhsT=v2_sb[kb][:, h0c], rhs=e[:, 0:QH],
                                start=first, stop=last,
                            )
                            nc.tensor.matmul(
                                pv1, lhsT=v2_sb[kb][:, h1c], rhs=e[:, QH : 2 * QH],
                                start=first, stop=last,
                            )
                            for thunk in hooks.get(kb, ()):
                                thunk()
                        # lazy normalization: free pv banks with two DVE copies,
                        # then swap den halves / reciprocal / scale off-path.
                        tmp0 = tmp_pool.tile([P, QH], F32, name="tmp0", tag="tmp0")
                        nc.vector.tensor_copy(tmp0, pv0)
                        tmp1 = tmp_pool.tile([P, QH], F32, name="tmp1", tag="tmp1")
                        nc.vector.tensor_copy(tmp1, pv1)
                        # tmp0 = [out_h0 ; den_h0], tmp1 = [den_h1 ; out_h1]
                        dsw = norm_pool.tile([P, QH], F32, name="dsw", tag="dsw")
                        nc.sync.dma_start(dsw[0:64, :], tmp0[64:128, :])
                        nc.sync.dma_start(dsw[64:128, :], tmp1[0:64, :])
                        rec = norm_pool.tile([P, QH], F32, name="rec", tag="rec")
                        nc.vector.reciprocal_approx_fast(rec, dsw)
                        nc.vector.tensor_tensor(
                            onorm[pr][0:64, qsl], tmp0[0:64, :], rec[0:64, :],
                            mybir.AluOpType.mult,
                        )
                        nc.vector.tensor_tensor(
                            onorm[pr][64:128, qsl], tmp1[64:128, :], rec[64:128, :],
                            mybir.AluOpType.mult,
                        )

                    # ---------- prologue ----------
                    # only work whose inputs arrive in DMA wave 1 (c0 + weights)
                    proj_qk(0, 0, "q")
                    proj_qk(0, 0, "k")
                    proj_v(0)
                    proj_v(1)

                    # ---------- deferred-work schedule ----------
                    # stripe(0,0): just-in-time V production, KT chunks hooked
                    # after their xt DMA wave lands (avoids stalling the
                    # in-order tensor queue on xt c1..c3 arrival), + QT(0,c1)
                    hooks_00 = {kb: [lambda kb=kb: proj_v(kb + 2)] for kb in range(KB - 2)}
                    hooks_00[0].append(lambda: proj_qk(0, 1, "k"))
                    hooks_00[4].append(lambda: proj_qk(0, 2, "k"))
                    hooks_00[8].append(lambda: proj_qk(0, 3, "k"))
                    hooks_00[KB - 2] = [lambda: proj_qk(0, 1, "q")]

                    def mk(pr_c_kind_list):
                        """Spread thunks across a stripe at kb = 4, 9, 14."""
                        slots = [4, 9, 14]
                        h = {}
                        for i, (pr_, c_, kind_) in enumerate(pr_c_kind_list):
                            h.setdefault(slots[i % 3], []).append(
                                lambda pr_=pr_, c_=c_, kind_=kind_: proj_qk(pr_, c_, kind_)
                            )
                        return h

                    hook_plan = {
                        (0, 0): hooks_00,
                        (0, 1): mk([(0, 2, "q"), (1, 0, "q"), (1, 0, "k")]),
                        (0, 2): mk([(0, 3, "q"), (1, 1, "k"), (1, 2, "k")]),
                        (0, 3): mk([(1, 3, "k"), (1, 1, "q")]),
                        (1, 0): mk([(1, 2, "q"), (2, 0, "k")]),
                        (1, 1): mk([(1, 3, "q"), (2, 1, "k")]),
                        (1, 2): mk([(2, 2, "k"), (2, 0, "q")]),
                        (1, 3): mk([(2, 3, "k"), (2, 1, "q")]),
                        (2, 0): mk([(2, 2, "q"), (3, 0, "k")]),
                        (2, 1): mk([(2, 3, "q"), (3, 1, "k")]),
                        (2, 2): mk([(3, 2, "k"), (3, 0, "q")]),
                        (2, 3): mk([(3, 3, "k"), (3, 1, "q")]),
                        (3, 0): mk([(3, 2, "q")]),
                        (3, 1): mk([(3, 3, "q")]),
                        (3, 2): {},
                        (3, 3): {},
                    }

                    # ---------- main loop ----------
                    # phase3 for stripe qh is spread as hooks inside stripe
                    # (3, qh+1); the final stripe's chunks run in the tail.
                    for pr in range(FT):
                        for qh in range(NQH):
                            hooks = {k: list(v) for k, v in hook_plan[(pr, qh)].items()}
                            if pr == FT - 1 and qh > 0:
                                for dt in range(DT):
                                    hooks.setdefault(2 * dt + 1, []).append(
                                        lambda dt=dt, c=qh - 1: phase3_chunk(dt, c)
                                    )
                            if pr == FT - 1 and qh == NQH - 1:
                                for dt in range(DT):
                                    hooks.setdefault(2 * dt, []).append(
                                        lambda dt=dt: phase3_pre(dt)
                                    )
                            stripe(pr, qh, hooks)
                    for dt in range(DT):
                        phase3_post(dt)

    return nc


_CACHE: dict = {}


def _get_nc(seq: int = S) -> bass.Bass:
    key = f"nc{seq}"
    if key not in _CACHE:
        nc = build_attention_nc(seq)
        nc.finalize()
        _CACHE[key] = nc
    return _CACHE[key]


def make_in_maps(x, Wq, bq, Wk, Wv, bv, Wo, seq: int = S):
    bf = ml_dtypes.bfloat16
    scale = 1.0 / math.sqrt(DK)
    x = np.asarray(x, np.float32)
    Wq = np.asarray(Wq, np.float32)
    bq = np.asarray(bq, np.float32)
    Wk = np.asarray(Wk, np.float32)
    Wv = np.asarray(Wv, np.float32)
    bv = np.asarray(bv, np.float32)
    Wo = np.asarray(Wo, np.float32)
    in_maps = []
    for core in range(NCORES):
        b, g = divmod(core, GROUPS)
        gsl = slice(g * DG, (g + 1) * DG)
        in_maps.append(
            {
                "xt": np.ascontiguousarray(x[b, :seq, :].T).astype(bf),
                "wqt": np.ascontiguousarray((Wq[gsl, :] * scale).T).astype(bf),
                "wkt": np.ascontiguousarray(Wk[gsl, :].T).astype(bf),
                "wvt": np.ascontiguousarray(Wv[gsl, :].T).astype(bf),
                "wot": np.ascontiguousarray(Wo[:, gsl].T).astype(bf),
                "bqs": np.ascontiguousarray(
                    (bq[gsl] * scale).astype(np.float32).reshape(FT, P).T
                ),
            }
        )
    return in_maps


def run_device(in_maps, seq: int = S, trace: bool = False):
    nc = _get_nc(seq)
    return run_bass_kernel_spmd(nc, in_maps, list(range(NCORES)), trace=trace)


def kernel(x, Wq, bq, Wk, bk, Wv, bv, Wo, bo):
    in_maps = make_in_maps(x, Wq, bq, Wk, Wv, bv, Wo)
    res = run_device(in_maps).results
    # V-bias folds into the output bias because softmax weights sum to 1:
    # softmax(s) @ (V + 1 bv^T) @ Wo.T + bo = softmax(s) @ V @ Wo.T + (bo + Wo @ bv)
    bo_eff = np.asarray(bo, np.float32) + np.asarray(Wo, np.float32) @ np.asarray(
        bv, np.float32
    )
    out = np.empty((B, S, D), np.float32)
    for b in range(B):
        acc = res[2 * b]["out"].astype(np.float32) + res[2 * b + 1]["out"].astype(
            np.float32
        )
        out[b] = acc.T + bo_eff[None, :]
    return out
